# revision 27
# baseline (speedup 1.0000x reference)
"""Multi-head attention (B=4, S=2048, D=1024, H=16, HD=64) on 8 TRN2 NeuronCores.

Sharding: core c handles batch b=c//2 and head-group g=c%2 (8 heads).
W_q/W_k/W_v column-sharded, W_o row-sharded; the two partial outputs per
batch are summed on the host.

v3 vs v2 baseline:
  - q/k/v projections run as fp8e4 DoubleRow matmuls with hi/lo error
    compensation: x and W are split host-side into fp8 hi + fp8 lo
    (pre-scaled by 16/64 to keep the lo terms out of fp8 denormals) and
    the product is computed as (Wh+Wl)'Xh + Wh'Xl (the dropped Xl*Wl
    term is ~0.1% incoherent noise, below bf16 round-off). 12 DR
    matmuls replace 8 bf16 matmuls per unit: 0.75x moving columns at
    0.5 cycles/row = ~2.7x faster projections, slightly *better* than
    bf16 accuracy (measured q rel err 1.1e-3 vs bf16 2.9e-3).
  - ctx is computed transposed: out[i-chunk, hd+1] = at[j,i-chunk]' @
    v[j, hd+1] per 128-i-chunk, M=128 instead of 65 -> ~0.51x moving
    columns. Denominators land per-partition (col 64), so softmax
    normalization is one strided reciprocal + per-partition
    tensor_scalar mul -- the v2 DRAM-bounce partition-broadcast chain
    is gone. A PE transpose (through a bitcast view of a projp PSUM
    slot) restores the f-major ctxT layout the out-projection wants.
  - scores / exp / causal-mask multiplies / out-projection unchanged
    (fp8 anywhere in the q/k/at/v value paths measurably breaks the
    2e-2 budget: attention-weight noise passes straight into ctx for
    incoherent v).
"""

import sys

sys.path.insert(0, "/opt/trn_rl_repo")

import numpy as np
import ml_dtypes

import concourse.bacc as bacc
import concourse.tile as tile
from concourse import mybir

BF16 = ml_dtypes.bfloat16
F8 = ml_dtypes.float8_e4m3  # TRN fp8e4: max normal 240, then inf
F32 = mybir.dt.float32
BF = mybir.dt.bfloat16
FP8 = mybir.dt.float8e4

B, S, D, H, HD = 4, 2048, 1024, 16, 64
G = 2              # head groups (cores per batch)
HPG = H // G       # 8 heads per group
NPAIR = HPG // 2   # 4 head pairs
FB = HPG * HD      # 512 projection cols per group
BLK = 128          # j-tile size
IBW = 512          # i-block width
NIB = S // IBW     # 4 i-blocks
NJT = S // BLK     # 16 j-tiles
NDT = D // BLK     # 8 contraction tiles
NST = S // BLK     # 16 s-tiles for the output projection
VW = HD + 1        # 65: v plus ones column
NCH = IBW // BLK   # 4 i-chunks per i-block
CRW = 128          # ctx psum region stride (f32 cols; 512B, no bank cross)
EXP_SCALE = 1.0 / np.sqrt(np.float32(HD))
SX = 16.0          # host pre-scale on x before fp8 hi/lo split
SW = 64.0          # host pre-scale on W_{q,k,v} before fp8 hi/lo split
SWO = 64.0         # host pre-scale on W_o before fp8 hi/lo split
INV_SCALE = 1.0 / (SX * SW)
DR = mybir.MatmulPerfMode.DoubleRow


def classify_mask(mask: np.ndarray):
    """Block states over the *transposed* mask grid: state[jt][it]:
    0=all valid, 1=all masked, 2=mixed."""
    m = np.asarray(mask)
    blocks = m.reshape(NJT, BLK, NJT, BLK).transpose(0, 2, 1, 3)  # [it, jt, i, j]
    anym = blocks.any(axis=(2, 3))
    allm = blocks.all(axis=(2, 3))
    states = np.where(allm, 1, np.where(anym, 2, 0)).astype(np.int8)
    return states.T  # index [jt, it]


def build_plan(states: np.ndarray):
    """Per i-block: list of (jt, c0, c1, mixed_ks)."""
    plan = []
    mixed_slots = {}
    for ib in range(NIB):
        its = list(range(4 * ib, 4 * ib + 4))
        jts = []
        for jt in range(NJT):
            sub = [int(states[jt, it]) for it in its]
            nz = [k for k, st in enumerate(sub) if st != 1]
            if not nz:
                continue
            k0, k1 = nz[0], nz[-1]
            mixed = [k for k in range(k0, k1 + 1) if sub[k] != 0]
            for k in mixed:
                mixed_slots.setdefault((jt, its[k]), len(mixed_slots))
            jts.append((jt, k0 * BLK, (k1 + 1) * BLK, mixed))
        assert jts, "fully-masked i-block not supported"
        plan.append(jts)
    return plan, mixed_slots


def plan_key(plan, mixed_slots):
    return (
        tuple(
            tuple((jt, c0, c1, tuple(mk)) for jt, c0, c1, mk in jts) for jts in plan
        ),
        tuple(sorted(mixed_slots.items())),
    )


def build_nc(plan, mixed_slots, reps=1):
    nvb = max(1, len(mixed_slots))
    nc = bacc.Bacc("TRN2", target_bir_lowering=False, debug=False, num_devices=8)

    # x: fp8 hi/lo interleaved, layout [p, d, 2, s]; w same with f cols
    xq8 = nc.dram_tensor("xq8", [BLK, NDT * 2 * S], FP8, kind="ExternalInput").ap()
    xk8 = nc.dram_tensor("xk8", [BLK, NDT * 2 * S], FP8, kind="ExternalInput").ap()
    xv8 = nc.dram_tensor("xv8", [BLK, NDT * 2 * S], FP8, kind="ExternalInput").ap()
    wq8 = nc.dram_tensor("wq8", [BLK, NDT * 2 * FB], FP8, kind="ExternalInput").ap()
    wk8 = nc.dram_tensor("wk8", [BLK, NDT * 2 * FB], FP8, kind="ExternalInput").ap()
    wv8 = nc.dram_tensor("wv8", [BLK, NDT * 2 * FB], FP8, kind="ExternalInput").ap()
    wo = nc.dram_tensor("wo", [FB, D], BF, kind="ExternalInput").ap()
    ident = nc.dram_tensor("ident", [BLK, BLK], BF, kind="ExternalInput").ap()
    validT = nc.dram_tensor("validT", [nvb, BLK, BLK], BF, kind="ExternalInput").ap()
    out = nc.dram_tensor("out", [S, D], F32, kind="ExternalOutput").ap()

    with tile.TileContext(nc) as tc:
        import contextlib

        ctxmgr = contextlib.ExitStack()
        with ctxmgr:
            persist = ctxmgr.enter_context(tc.tile_pool(name="persist", bufs=1))
            xpool = ctxmgr.enter_context(tc.tile_pool(name="xpool", bufs=2))
            scp = ctxmgr.enter_context(tc.tile_pool(name="scp", bufs=2, space="PSUM"))
            projp = ctxmgr.enter_context(tc.tile_pool(name="projp", bufs=2, space="PSUM"))
            ctxp = ctxmgr.enter_context(tc.tile_pool(name="ctxp", bufs=2, space="PSUM"))
            atp = ctxmgr.enter_context(tc.tile_pool(name="atp", bufs=4))
            small = ctxmgr.enter_context(tc.tile_pool(name="small", bufs=4))
            stgp = ctxmgr.enter_context(tc.tile_pool(name="stgp", bufs=18))

            # ---- persistent tiles (allocated once; reps re-fill them) -----
            wv_t = persist.tile([BLK, NDT * 2 * FB], FP8, name="wv_t")
            wk_t = persist.tile([BLK, NDT * 2 * FB], FP8, name="wk_t")
            wq_t = persist.tile([BLK, NDT * 2 * FB], FP8, name="wq_t")
            wo_t = persist.tile([BLK, NPAIR * D], BF, name="wo_t")
            ident_t = persist.tile([BLK, BLK], BF, name="ident_t")
            valid_sb = persist.tile([BLK, nvb * BLK], BF, name="valid_sb")
            qT_sb = [persist.tile([BLK, S], BF, name=f"qT{p}") for p in range(NPAIR)]
            kT_sb = [persist.tile([BLK, S], BF, name=f"kT{p}") for p in range(NPAIR)]
            v_sb = [persist.tile([BLK, HPG * VW], BF, name=f"v{j}") for j in range(NJT)]
            ctxT_sb = [persist.tile([BLK, S], BF, name=f"cT{p}") for p in range(NPAIR)]

            for rep in range(reps):
                emit_body(
                    nc, plan, mixed_slots, nvb,
                    xq8, xk8, xv8, wq8, wk8, wv8, wo, ident, validT, out,
                    wv_t, wk_t, wq_t, wo_t, ident_t, valid_sb,
                    qT_sb, kT_sb, v_sb, ctxT_sb,
                    xpool, scp, projp, ctxp, atp, small, stgp, rep,
                )

    nc.compile()
    return nc


def emit_body(
    nc, plan, mixed_slots, nvb,
    xq8, xk8, xv8, wq8, wk8, wv8, wo, ident, validT, out,
    wv_t, wk_t, wq_t, wo_t, ident_t, valid_sb,
    qT_sb, kT_sb, v_sb, ctxT_sb,
    xpool, scp, projp, ctxp, atp, small, stgp, rep,
):
    r = f"r{rep}"

    # DRAM views: x as [p, d, 2, s], weights as [p, d, 2, f]
    xv_v = xv8.rearrange("p (d two s) -> p d two s", two=2, s=S)
    xk_v = xk8.rearrange("p (d two s) -> p d two s", two=2, s=S)
    xq_v = xq8.rearrange("p (d two s) -> p d two s", two=2, s=S)

    # ---- input DMAs: one per (tensor, ib) + one per weight ------------
    # Order: v-side first (vproj gates everything), then k/q for ib0 so
    # the first attention can start after ~6 MB, then the rest.
    qs = [nc.sync, nc.scalar]

    def big(i, dst, src):
        # split along d so dependent matmuls can start on the first half
        qs[i % 2].dma_start(out=dst[:, 0:NDT // 2], in_=src[:, 0:NDT // 2])
        qs[(i + 1) % 2].dma_start(out=dst[:, NDT // 2:], in_=src[:, NDT // 2:])

    xv_c, xk_c, xq_c = [], [], []

    def load_x(ib, i0):
        cs = slice(ib * IBW, (ib + 1) * IBW)
        for i, (lst, nm, v) in enumerate(
            ((xv_c, "xv", xv_v), (xk_c, "xk", xk_v), (xq_c, "xq", xq_v))
        ):
            t = xpool.tile([BLK, NDT * 2 * IBW], FP8, tag=nm, name=f"{nm}_{ib}_{r}")
            lst.append(t.rearrange("p (d two s) -> p d two s", two=2, s=IBW))
            big(i0 + i, lst[ib], v[:, :, :, cs])

    w_shape = "p (d two f) -> p d two f"
    wv_v = wv_t.rearrange(w_shape, two=2, f=FB)
    wk_v = wk_t.rearrange(w_shape, two=2, f=FB)
    wq_v = wq_t.rearrange(w_shape, two=2, f=FB)
    wv8_v = wv8.rearrange(w_shape, two=2, f=FB)
    wk8_v = wk8.rearrange(w_shape, two=2, f=FB)
    wq8_v = wq8.rearrange(w_shape, two=2, f=FB)
    H1, H2 = slice(0, NDT // 2), slice(NDT // 2, NDT)
    Q = [slice(2 * i, 2 * i + 2) for i in range(4)]
    # startup-critical: first halves of wv+xv0 land first so the first
    # vproj's d0-3 matmuls start ~3us earlier; then k/q for ib0.
    cs0 = slice(0, IBW)
    xt0 = []
    for nm, lst in (("xv", xv_c), ("xk", xk_c), ("xq", xq_c)):
        t = xpool.tile([BLK, NDT * 2 * IBW], FP8, tag=nm, name=f"{nm}_0_{r}")
        lst.append(t.rearrange("p (d two s) -> p d two s", two=2, s=IBW))
        xt0.append(lst[0])
    for q in Q:
        nc.sync.dma_start(out=wv_v[:, q], in_=wv8_v[:, q])
        nc.scalar.dma_start(out=xv_c[0][:, q], in_=xv_v[:, q, :, cs0])
    nc.sync.dma_start(out=wk_v[:, H1], in_=wk8_v[:, H1])
    nc.scalar.dma_start(out=xk_c[0][:, H1], in_=xk_v[:, H1, :, cs0])
    nc.sync.dma_start(out=wk_v[:, H2], in_=wk8_v[:, H2])
    nc.scalar.dma_start(out=xk_c[0][:, H2], in_=xk_v[:, H2, :, cs0])
    nc.sync.dma_start(out=wq_v[:, H1], in_=wq8_v[:, H1])
    nc.scalar.dma_start(out=xq_c[0][:, H1], in_=xq_v[:, H1, :, cs0])
    nc.sync.dma_start(out=wq_v[:, H2], in_=wq8_v[:, H2])
    nc.scalar.dma_start(out=xq_c[0][:, H2], in_=xq_v[:, H2, :, cs0])
    nc.scalar.dma_start(out=ident_t, in_=ident)
    if nvb:
        nc.scalar.dma_start(
            out=valid_sb.rearrange("p (n f) -> p n f", f=BLK),
            in_=validT.rearrange("n p f -> p n f"),
        )
    for ib in range(1, NIB):
        load_x(ib, ib)
    nc.scalar.dma_start(
        out=wo_t.rearrange("p (q e) -> p q e", e=D),
        in_=wo.rearrange("(q p) e -> p q e", p=BLK),
    )

    # ---- compute helpers ---------------------------------------------
    # hi/lo fp8 DoubleRow product: acc += (Ah+Al)' Bh  +  Ah' Bl
    #   A = stationary [p, d, 2, m-cols], B = moving [p, d, 2, n-cols]
    def dr_matmuls(ps, a4, b4, mslice, nslice):
        for d in range(NDT):
            nc.tensor.matmul(
                ps,
                a4[:, d, :, mslice],
                b4[:, d, 0:1, nslice].broadcast_to(
                    (BLK, 2, b4[:, d, 0, nslice].shape[-1])
                ),
                start=(d == 0),
                stop=False,
                perf_mode=DR,
            )
        for d0 in range(0, NDT, 2):
            nc.tensor.matmul(
                ps,
                a4[:, d0:d0 + 2, 0, mslice],
                b4[:, d0:d0 + 2, 1, nslice],
                start=False,
                stop=(d0 == NDT - 2),
                perf_mode=DR,
            )

    def vproj_unit(j):
        sb = j // 4
        jc = slice((j % 4) * BLK, (j % 4 + 1) * BLK)
        ps = projp.tile([BLK, IBW], F32, tag="pp", name=f"vps{j}_{r}")
        wv4 = wv_t.rearrange(w_shape, two=2, f=FB)
        dr_matmuls(ps, xv_c[sb], wv4, jc, slice(0, FB))
        dst = v_sb[j].rearrange("p (h w) -> p h w", w=VW)
        nc.vector.tensor_scalar_mul(
            dst[:, :, 0:HD], ps.rearrange("p (h w) -> p h w", w=HD), INV_SCALE
        )
        nc.vector.memset(dst[:, :, HD:VW], 1.0)

    def project(p, ib, w_t, x_c, dst_sb, nm):
        ps = projp.tile([BLK, IBW], F32, tag="pp", name=nm)
        w4 = w_t.rearrange(w_shape, two=2, f=FB)
        dr_matmuls(ps, w4, x_c[ib], slice(p * BLK, (p + 1) * BLK), slice(0, IBW))
        nc.vector.tensor_scalar_mul(
            dst_sb[p][:, ib * IBW:(ib + 1) * IBW], ps, INV_SCALE
        )

    filler = []
    UNIT_COST = {"ctxT": 1}

    def drain(budget):
        while filler and budget > 0:
            k, fn = filler.pop(0)
            fn()
            budget -= UNIT_COST.get(k, 4)

    def drain_kind(kind):
        keep = []
        for k, fn in filler:
            if k == kind:
                fn()
            else:
                keep.append((k, fn))
        filler[:] = keep

    def attention(p, ib):
        jts = plan[ib]
        nj = len(jts)
        # coverage bookkeeping: which entries touch each 128-i-chunk
        cov = [
            [e for e, (jt, c0, c1, mx) in enumerate(jts) if c0 <= ch * BLK < c1]
            for ch in range(NCH)
        ]
        last_cov = [c[-1] for c in cov]
        # ctx psum: one 1-bank tile per head, [p, chunk(4), 128] with data
        # in [0:65] of each region; bufs=2 double-buffers across head pairs.
        ctx_ps = [
            ctxp.tile([BLK, NCH * CRW], F32, tag="ctx", name=f"c_{p}_{ib}_{h}_{r}")
            for h in range(2)
        ]
        ctx4 = [t.rearrange("p (c v) -> p c v", c=NCH, v=CRW) for t in ctx_ps]
        bank_started = [False, False]
        sc_t = {}

        def emit_scores(e):
            jt, c0, c1, mixed = jts[e]
            w = c1 - c0
            sc = scp.tile([BLK, 2 * IBW], F32, tag="sc", name=f"s{p}_{ib}_{jt}_{r}")
            nc.tensor.matmul(
                sc[:, c0:c1],
                kT_sb[p][0:HD, jt * BLK:(jt + 1) * BLK],
                qT_sb[p][0:HD, ib * IBW + c0:ib * IBW + c1],
                start=True,
                stop=True,
            )
            nc.tensor.matmul(
                sc[:, IBW:IBW + w],
                kT_sb[p][HD:BLK, jt * BLK:(jt + 1) * BLK],
                qT_sb[p][HD:BLK, ib * IBW + c0:ib * IBW + c1],
                start=True,
                stop=True,
                tile_position=(HD, 0),
            )
            for k in mixed:
                slot = mixed_slots[(jt, 4 * ib + k)]
                mv = valid_sb[:, slot * BLK:(slot + 1) * BLK]
                nc.tensor.matmul(
                    sc[:, k * BLK:(k + 1) * BLK],
                    ident_t, mv,
                    start=False, stop=True, skip_group_check=True,
                )
                h1c = IBW + k * BLK - c0
                nc.tensor.matmul(
                    sc[:, h1c:h1c + BLK],
                    ident_t, mv,
                    start=False, stop=True, skip_group_check=True,
                )
            sc_t[e] = sc

        at_t = {}

        def emit_exp(e):
            jt, c0, c1, mixed = jts[e]
            w = c1 - c0
            sc = sc_t.pop(e)
            at = atp.tile([BLK, 2 * IBW], BF, tag="at", name=f"a{p}_{ib}_{jt}_{r}")
            nc.scalar.activation(
                out=at[:, c0:IBW + w],
                in_=sc[:, c0:IBW + w],
                func=mybir.ActivationFunctionType.Exp,
                scale=float(EXP_SCALE),
            )
            at_t[e] = at

        def emit_ctx(e):
            jt, c0, c1, mixed = jts[e]
            at = at_t.pop(e)
            vv = v_sb[jt].rearrange("p (h w) -> p h w", w=VW)
            for h in range(2):
                for ch in range(c0 // BLK, NCH):
                    if h == 0:
                        lhsT = at[:, ch * BLK:(ch + 1) * BLK]
                    else:
                        o = IBW + ch * BLK - c0
                        lhsT = at[:, o:o + BLK]
                    st = not bank_started[h]
                    bank_started[h] = True
                    nc.tensor.matmul(
                        ctx4[h][:, ch, 0:VW],
                        lhsT,
                        vv[:, 2 * p + h, :],
                        start=st,
                        stop=(e == last_cov[ch]),
                    )

        # Per 2-entry group: exps first (registers the sc-slot readers),
        # one filler unit (~independent matmuls) to absorb the exp wait,
        # then the next score pair (WAR on the sc slot), then both ctx
        # batches.
        emit_scores(0)
        if nj > 1:
            emit_scores(1)
        for e0 in range(0, nj, 2):
            es = [e for e in (e0, e0 + 1) if e < nj]
            for e in es:
                emit_exp(e)
            drain(8)
            for e in es:
                if e + 2 < nj:
                    emit_scores(e + 2)
            for e in es:
                emit_ctx(e)

        # evacuation: one strided reciprocal of the 8 denominator columns,
        # then per (head, chunk): per-partition normalize -> bf16 stg ->
        # PE transpose (bitcast view of a projp slot) -> Pool copy into
        # the f-major ctxT tile the out-projection consumes.
        rec = small.tile([BLK, 2 * NCH], F32, tag="rec", name=f"rc{p}_{ib}_{r}")
        rec3 = rec.rearrange("p (h c v) -> p h c v", h=2, c=NCH, v=1)
        for h in range(2):
            nc.vector.reciprocal(out=rec3[:, h], in_=ctx4[h][:, :, HD:VW])
        for h in range(2):
            # one normalize-mul per head: recip column broadcast (stride 0)
            # along hd, all 4 chunks in one op
            stg = stgp.tile([BLK, NCH * HD], BF, tag="stg", name=f"st{p}_{ib}_{h}_{r}")
            stg3 = stg.rearrange("p (c w) -> p c w", w=HD)
            nc.vector.tensor_mul(
                stg3,
                ctx4[h][:, :, 0:HD],
                rec3[:, h].broadcast_to((BLK, NCH, HD)),
            )
            for ch in range(NCH):
                # transpose + copy only read stg, not the ctx psum: defer
                # them into the filler stream so the PE transpose never
                # stalls inline on the DVE normalize.
                def tposeu(stg_=stg3, h_=h, ch_=ch, p_=p, ib_=ib):
                    tp = projp.tile([BLK, IBW], F32, tag="pp",
                                    name=f"tp{p_}_{ib_}_{h_}_{ch_}_{r}")
                    tpv = tp[:, :].bitcast(BF)
                    nc.tensor.transpose(tpv[0:HD, 0:BLK], stg_[:, ch_], ident_t)
                    nc.vector.tensor_copy(
                        ctxT_sb[p_][h_ * HD:(h_ + 1) * HD,
                                    ib_ * IBW + ch_ * BLK:ib_ * IBW + (ch_ + 1) * BLK],
                        tpv[0:HD, 0:BLK],
                    )

                filler.append(("ctxT", tposeu))

    def outproj_mms(po, st, nb, ps):
        for p in ps:
            nc.tensor.matmul(
                po,
                ctxT_sb[p][:, st * BLK:(st + 1) * BLK],
                wo_t[:, p * D + nb * IBW:p * D + (nb + 1) * IBW],
                start=(p == 0),
                stop=(p == NPAIR - 1),
            )

    def outproj_evac(po, st, nb):
        ot = small.tile([BLK, IBW], F32, tag="ot", name=f"ot{st}_{nb}_{r}")
        nc.vector.tensor_copy(ot, po)
        nc.scalar.dma_start(
            out=out[st * BLK:(st + 1) * BLK, nb * IBW:(nb + 1) * IBW],
            in_=ot,
        )

    def outproj_unit(st, nb):
        po = projp.tile([BLK, IBW], F32, tag="pp", name=f"po{st}_{nb}_{r}")
        outproj_mms(po, st, nb, range(NPAIR))
        outproj_evac(po, st, nb)

    def outproj_q(st, nq):
        # quarter-width unit (256 out cols) for the kernel tail
        pof = projp.tile([BLK, IBW], F32, tag="pp", name=f"pq{st}_{nq}_{r}")
        po = pof[:, 0:IBW // 2]
        qc = slice(nq * (IBW // 2), (nq + 1) * (IBW // 2))
        for p in range(NPAIR):
            nc.tensor.matmul(
                po,
                ctxT_sb[p][:, st * BLK:(st + 1) * BLK],
                wo_t[:, p * D:(p + 1) * D][:, qc],
                start=(p == 0),
                stop=(p == NPAIR - 1),
            )
        otf = small.tile([BLK, IBW], F32, tag="ot", name=f"otq{st}_{nq}_{r}")
        ot = otf[:, 0:IBW // 2]
        nc.vector.tensor_copy(ot, po)
        nc.scalar.dma_start(
            out=out[st * BLK:(st + 1) * BLK, qc],
            in_=ot,
        )

    def outproj_halves(st, nb):
        # two 2-matmul filler units sharing one accumulation group; A and
        # B are pushed adjacently so the 2-slot pp rotation never has to
        # wait on a B that sits later in the PE FIFO.
        holder = []

        def ua():
            po = projp.tile([BLK, IBW], F32, tag="pp", name=f"po{st}_{nb}_{r}")
            holder.append(po)
            outproj_mms(po, st, nb, (0, 1))

        def ub():
            po = holder[0]
            outproj_mms(po, st, nb, (2, 3))
            outproj_evac(po, st, nb)

        return ua, ub

    # ---- main schedule ------------------------------------------------
    # vproj(ib0) runs inline before the first attention; later vprojs and
    # all out-projections flow through the filler queue, drained one unit
    # per 2-entry attention group (PE bubble absorption). vproj units are
    # force-drained at each ib boundary (needed by the next ib's ctx).
    for j in range(4):
        vproj_unit(j)
    for ib in range(NIB):
        for p in range(NPAIR):
            if p == 2 and ib + 1 < NIB:
                for j in range(4 * (ib + 1), 4 * (ib + 1) + 4):
                    filler.append(("vproj", (lambda jj: lambda: vproj_unit(jj))(j)))
            if p == 1 and ib > 0:
                for st in range(4 * (ib - 1), 4 * (ib - 1) + 4):
                    for nb in range(2):
                        if ib == NIB - 1:
                            # ib3 has 32 drain slots but little supply:
                            # split its outproj fillers into halves
                            ua, ub = outproj_halves(st, nb)
                            filler.append(("outproj", ua))
                            filler.append(("outproj", ub))
                        else:
                            filler.append(
                                ("outproj", (lambda s_, n_: lambda: outproj_unit(s_, n_))(st, nb))
                            )
            if p == 0 and ib == 0:
                project(0, 0, wk_t, xk_c, kT_sb, f"kps0_0_{r}")
                project(0, 0, wq_t, xq_c, qT_sb, f"qps0_0_{r}")
            # queue next pair's k/q projections as filler (emitted inside
            # this attention; force-drained at its end)
            np_, nib = (p + 1, ib) if p + 1 < NPAIR else (0, ib + 1)
            if nib < NIB:
                for w_t, x_c, dst, nm in (
                    (wk_t, xk_c, kT_sb, f"kps{np_}_{nib}_{r}"),
                    (wq_t, xq_c, qT_sb, f"qps{np_}_{nib}_{r}"),
                ):
                    # deadline-critical: next pair's scores stall on these,
                    # so they jump the filler queue
                    filler.insert(
                        len([u for u in filler if u[0] == "kqps"]),
                        ("kqps", (lambda a, b, c, d, e2, f2: lambda: project(a, b, c, d, e2, f2))(
                            np_, nib, w_t, x_c, dst, nm))
                    )
            attention(p, ib)
            drain_kind("kqps")
        drain_kind("vproj")
    drain_kind("fin")
    drain_kind("ctxT")
    drain_kind("outproj")
    for st in range(4 * (NIB - 1), 4 * (NIB - 1) + 4):
        for nb in range(2):
            outproj_unit(st, nb)



_BUILD_CACHE: dict = {}


def _get_nc(mask: np.ndarray, reps=1):
    states = classify_mask(mask)
    plan, mixed_slots = build_plan(states)
    key = (plan_key(plan, mixed_slots), reps)
    if key not in _BUILD_CACHE:
        _BUILD_CACHE[key] = (build_nc(plan, mixed_slots, reps), plan, mixed_slots)
    return _BUILD_CACHE[key]


def _hilo_dr(a, scale):
    """[R, C] f32 -> fp8 hi/lo interleaved [128, (R//128)*2*C]."""
    a = np.asarray(a, np.float32) * scale
    a = a.reshape(-1, BLK, a.shape[-1])            # [d, p, C]
    hi = np.clip(a, -240, 240).astype(F8)
    lo = np.clip(a - hi.astype(np.float32), -240, 240).astype(F8)
    arr = np.stack([hi, lo], axis=2)               # [d, p, 2, C]
    return np.ascontiguousarray(arr.transpose(1, 0, 2, 3)).reshape(BLK, -1)


def _make_in_maps(xq, xk, xv, mask, W_q, W_k, W_v, W_o, mixed_slots):
    nvb = max(1, len(mixed_slots))
    vt = np.zeros((nvb, BLK, BLK), BF16)
    m = np.asarray(mask)
    for (jt, it), slot in mixed_slots.items():
        blk = m[it * BLK:(it + 1) * BLK, jt * BLK:(jt + 1) * BLK]
        vt[slot] = (-30000.0 * blk.T.astype(np.float32)).astype(BF16)
    ident = np.eye(BLK, dtype=BF16)
    xT = {}
    for b in range(B):
        xT[b] = tuple(
            _hilo_dr(np.asarray(x[b]).T, SX) for x in (xq, xk, xv)
        )
    wmaps = {}
    for g in range(G):
        cols = slice(g * FB, (g + 1) * FB)
        wmaps[g] = (
            _hilo_dr(np.asarray(W_q)[:, cols], SW),
            _hilo_dr(np.asarray(W_k)[:, cols], SW),
            _hilo_dr(np.asarray(W_v)[:, cols], SW),
            np.asarray(W_o)[cols, :].astype(BF16),
        )
    in_maps = []
    for c in range(8):
        b, g = c // G, c % G
        in_maps.append(
            {
                "xq8": xT[b][0],
                "xk8": xT[b][1],
                "xv8": xT[b][2],
                "wq8": wmaps[g][0],
                "wk8": wmaps[g][1],
                "wv8": wmaps[g][2],
                "wo": wmaps[g][3],
                "ident": ident,
                "validT": vt,
            }
        )
    return in_maps


PROFILE = False
last_hw_exec_ns = None


def kernel(xq, xk, xv, mask, W_q, W_k, W_v, W_o):
    global last_hw_exec_ns
    from concourse import bass_utils

    nc, plan, mixed_slots = _get_nc(mask)
    in_maps = _make_in_maps(xq, xk, xv, mask, W_q, W_k, W_v, W_o, mixed_slots)
    res = bass_utils.run_bass_kernel_spmd(
        nc, in_maps, core_ids=list(range(8))
    )
    if res.exec_time_ns:
        last_hw_exec_ns = res.exec_time_ns
    out = np.empty((B, S, D), np.float32)
    for b in range(B):
        out[b] = res.results[2 * b]["out"] + res.results[2 * b + 1]["out"]
    return out


# revision 28
# speedup vs baseline: 1.0027x; 1.0027x over previous
"""Multi-head attention (B=4, S=2048, D=1024, H=16, HD=64) on 8 TRN2 NeuronCores.

Sharding: core c handles batch b=c//2 and head-group g=c%2 (8 heads).
W_q/W_k/W_v column-sharded, W_o row-sharded; the two partial outputs per
batch are summed on the host.

v3 vs v2 baseline:
  - q/k/v projections run as fp8e4 DoubleRow matmuls with hi/lo error
    compensation: x and W are split host-side into fp8 hi + fp8 lo
    (pre-scaled by 16/64 to keep the lo terms out of fp8 denormals) and
    the product is computed as (Wh+Wl)'Xh + Wh'Xl (the dropped Xl*Wl
    term is ~0.1% incoherent noise, below bf16 round-off). 12 DR
    matmuls replace 8 bf16 matmuls per unit: 0.75x moving columns at
    0.5 cycles/row = ~2.7x faster projections, slightly *better* than
    bf16 accuracy (measured q rel err 1.1e-3 vs bf16 2.9e-3).
  - ctx is computed transposed: out[i-chunk, hd+1] = at[j,i-chunk]' @
    v[j, hd+1] per 128-i-chunk, M=128 instead of 65 -> ~0.51x moving
    columns. Denominators land per-partition (col 64), so softmax
    normalization is one strided reciprocal + per-partition
    tensor_scalar mul -- the v2 DRAM-bounce partition-broadcast chain
    is gone. A PE transpose (through a bitcast view of a projp PSUM
    slot) restores the f-major ctxT layout the out-projection wants.
  - scores / exp / causal-mask multiplies / out-projection unchanged
    (fp8 anywhere in the q/k/at/v value paths measurably breaks the
    2e-2 budget: attention-weight noise passes straight into ctx for
    incoherent v).
"""

import sys

sys.path.insert(0, "/opt/trn_rl_repo")

import numpy as np
import ml_dtypes

import concourse.bacc as bacc
import concourse.tile as tile
from concourse import mybir

BF16 = ml_dtypes.bfloat16
F8 = ml_dtypes.float8_e4m3  # TRN fp8e4: max normal 240, then inf
F32 = mybir.dt.float32
BF = mybir.dt.bfloat16
FP8 = mybir.dt.float8e4

B, S, D, H, HD = 4, 2048, 1024, 16, 64
G = 2              # head groups (cores per batch)
HPG = H // G       # 8 heads per group
NPAIR = HPG // 2   # 4 head pairs
FB = HPG * HD      # 512 projection cols per group
BLK = 128          # j-tile size
IBW = 512          # i-block width
NIB = S // IBW     # 4 i-blocks
NJT = S // BLK     # 16 j-tiles
NDT = D // BLK     # 8 contraction tiles
NST = S // BLK     # 16 s-tiles for the output projection
VW = HD + 1        # 65: v plus ones column
NCH = IBW // BLK   # 4 i-chunks per i-block
CRW = 128          # ctx psum region stride (f32 cols; 512B, no bank cross)
EXP_SCALE = 1.0 / np.sqrt(np.float32(HD))
SX = 16.0          # host pre-scale on x before fp8 hi/lo split
SW = 64.0          # host pre-scale on W_{q,k,v} before fp8 hi/lo split
SWO = 64.0         # host pre-scale on W_o before fp8 hi/lo split
INV_SCALE = 1.0 / (SX * SW)
DR = mybir.MatmulPerfMode.DoubleRow


def classify_mask(mask: np.ndarray):
    """Block states over the *transposed* mask grid: state[jt][it]:
    0=all valid, 1=all masked, 2=mixed."""
    m = np.asarray(mask)
    blocks = m.reshape(NJT, BLK, NJT, BLK).transpose(0, 2, 1, 3)  # [it, jt, i, j]
    anym = blocks.any(axis=(2, 3))
    allm = blocks.all(axis=(2, 3))
    states = np.where(allm, 1, np.where(anym, 2, 0)).astype(np.int8)
    return states.T  # index [jt, it]


def build_plan(states: np.ndarray):
    """Per i-block: list of (jt, c0, c1, mixed_ks)."""
    plan = []
    mixed_slots = {}
    for ib in range(NIB):
        its = list(range(4 * ib, 4 * ib + 4))
        jts = []
        for jt in range(NJT):
            sub = [int(states[jt, it]) for it in its]
            nz = [k for k, st in enumerate(sub) if st != 1]
            if not nz:
                continue
            k0, k1 = nz[0], nz[-1]
            mixed = [k for k in range(k0, k1 + 1) if sub[k] != 0]
            for k in mixed:
                mixed_slots.setdefault((jt, its[k]), len(mixed_slots))
            jts.append((jt, k0 * BLK, (k1 + 1) * BLK, mixed))
        assert jts, "fully-masked i-block not supported"
        plan.append(jts)
    return plan, mixed_slots


def plan_key(plan, mixed_slots):
    return (
        tuple(
            tuple((jt, c0, c1, tuple(mk)) for jt, c0, c1, mk in jts) for jts in plan
        ),
        tuple(sorted(mixed_slots.items())),
    )


def build_nc(plan, mixed_slots, reps=1):
    nvb = max(1, len(mixed_slots))
    nc = bacc.Bacc("TRN2", target_bir_lowering=False, debug=False, num_devices=8)

    # x: fp8 hi/lo interleaved, layout [p, d, 2, s]; w same with f cols
    xq8 = nc.dram_tensor("xq8", [BLK, NDT * 2 * S], FP8, kind="ExternalInput").ap()
    xk8 = nc.dram_tensor("xk8", [BLK, NDT * 2 * S], FP8, kind="ExternalInput").ap()
    xv8 = nc.dram_tensor("xv8", [BLK, NDT * 2 * S], FP8, kind="ExternalInput").ap()
    wq8 = nc.dram_tensor("wq8", [BLK, NDT * 2 * FB], FP8, kind="ExternalInput").ap()
    wk8 = nc.dram_tensor("wk8", [BLK, NDT * 2 * FB], FP8, kind="ExternalInput").ap()
    wv8 = nc.dram_tensor("wv8", [BLK, NDT * 2 * FB], FP8, kind="ExternalInput").ap()
    wo = nc.dram_tensor("wo", [FB, D], BF, kind="ExternalInput").ap()
    ident = nc.dram_tensor("ident", [BLK, BLK], BF, kind="ExternalInput").ap()
    validT = nc.dram_tensor("validT", [nvb, BLK, BLK], BF, kind="ExternalInput").ap()
    out = nc.dram_tensor("out", [S, D], F32, kind="ExternalOutput").ap()

    with tile.TileContext(nc) as tc:
        import contextlib

        ctxmgr = contextlib.ExitStack()
        with ctxmgr:
            persist = ctxmgr.enter_context(tc.tile_pool(name="persist", bufs=1))
            xpool = ctxmgr.enter_context(tc.tile_pool(name="xpool", bufs=2))
            scp = ctxmgr.enter_context(tc.tile_pool(name="scp", bufs=2, space="PSUM"))
            projp = ctxmgr.enter_context(tc.tile_pool(name="projp", bufs=2, space="PSUM"))
            ctxp = ctxmgr.enter_context(tc.tile_pool(name="ctxp", bufs=2, space="PSUM"))
            atp = ctxmgr.enter_context(tc.tile_pool(name="atp", bufs=4))
            small = ctxmgr.enter_context(tc.tile_pool(name="small", bufs=4))
            stgp = ctxmgr.enter_context(tc.tile_pool(name="stgp", bufs=18))

            # ---- persistent tiles (allocated once; reps re-fill them) -----
            wv_t = persist.tile([BLK, NDT * 2 * FB], FP8, name="wv_t")
            wk_t = persist.tile([BLK, NDT * 2 * FB], FP8, name="wk_t")
            wq_t = persist.tile([BLK, NDT * 2 * FB], FP8, name="wq_t")
            wo_t = persist.tile([BLK, NPAIR * D], BF, name="wo_t")
            ident_t = persist.tile([BLK, BLK], BF, name="ident_t")
            valid_sb = persist.tile([BLK, nvb * BLK], BF, name="valid_sb")
            qT_sb = [persist.tile([BLK, S], BF, name=f"qT{p}") for p in range(NPAIR)]
            kT_sb = [persist.tile([BLK, S], BF, name=f"kT{p}") for p in range(NPAIR)]
            v_sb = [persist.tile([BLK, HPG * VW], BF, name=f"v{j}") for j in range(NJT)]
            ctxT_sb = [persist.tile([BLK, S], BF, name=f"cT{p}") for p in range(NPAIR)]

            for rep in range(reps):
                emit_body(
                    nc, plan, mixed_slots, nvb,
                    xq8, xk8, xv8, wq8, wk8, wv8, wo, ident, validT, out,
                    wv_t, wk_t, wq_t, wo_t, ident_t, valid_sb,
                    qT_sb, kT_sb, v_sb, ctxT_sb,
                    xpool, scp, projp, ctxp, atp, small, stgp, rep,
                )

    nc.compile()
    return nc


def emit_body(
    nc, plan, mixed_slots, nvb,
    xq8, xk8, xv8, wq8, wk8, wv8, wo, ident, validT, out,
    wv_t, wk_t, wq_t, wo_t, ident_t, valid_sb,
    qT_sb, kT_sb, v_sb, ctxT_sb,
    xpool, scp, projp, ctxp, atp, small, stgp, rep,
):
    r = f"r{rep}"

    # DRAM views: x as [p, d, 2, s], weights as [p, d, 2, f]
    xv_v = xv8.rearrange("p (d two s) -> p d two s", two=2, s=S)
    xk_v = xk8.rearrange("p (d two s) -> p d two s", two=2, s=S)
    xq_v = xq8.rearrange("p (d two s) -> p d two s", two=2, s=S)

    # ---- input DMAs: one per (tensor, ib) + one per weight ------------
    # Order: v-side first (vproj gates everything), then k/q for ib0 so
    # the first attention can start after ~6 MB, then the rest.
    qs = [nc.sync, nc.scalar]

    def big(i, dst, src):
        # split along d so dependent matmuls can start on the first half
        qs[i % 2].dma_start(out=dst[:, 0:NDT // 2], in_=src[:, 0:NDT // 2])
        qs[(i + 1) % 2].dma_start(out=dst[:, NDT // 2:], in_=src[:, NDT // 2:])

    xv_c, xk_c, xq_c = [], [], []

    def load_x(ib, i0):
        cs = slice(ib * IBW, (ib + 1) * IBW)
        for i, (lst, nm, v) in enumerate(
            ((xv_c, "xv", xv_v), (xk_c, "xk", xk_v), (xq_c, "xq", xq_v))
        ):
            t = xpool.tile([BLK, NDT * 2 * IBW], FP8, tag=nm, name=f"{nm}_{ib}_{r}")
            lst.append(t.rearrange("p (d two s) -> p d two s", two=2, s=IBW))
            big(i0 + i, lst[ib], v[:, :, :, cs])

    w_shape = "p (d two f) -> p d two f"
    wv_v = wv_t.rearrange(w_shape, two=2, f=FB)
    wk_v = wk_t.rearrange(w_shape, two=2, f=FB)
    wq_v = wq_t.rearrange(w_shape, two=2, f=FB)
    wv8_v = wv8.rearrange(w_shape, two=2, f=FB)
    wk8_v = wk8.rearrange(w_shape, two=2, f=FB)
    wq8_v = wq8.rearrange(w_shape, two=2, f=FB)
    H1, H2 = slice(0, NDT // 2), slice(NDT // 2, NDT)
    Q = [slice(2 * i, 2 * i + 2) for i in range(4)]
    # startup-critical: first halves of wv+xv0 land first so the first
    # vproj's d0-3 matmuls start ~3us earlier; then k/q for ib0.
    cs0 = slice(0, IBW)
    xt0 = []
    for nm, lst in (("xv", xv_c), ("xk", xk_c), ("xq", xq_c)):
        t = xpool.tile([BLK, NDT * 2 * IBW], FP8, tag=nm, name=f"{nm}_0_{r}")
        lst.append(t.rearrange("p (d two s) -> p d two s", two=2, s=IBW))
        xt0.append(lst[0])
    for q in Q:
        nc.sync.dma_start(out=wv_v[:, q], in_=wv8_v[:, q])
        nc.scalar.dma_start(out=xv_c[0][:, q], in_=xv_v[:, q, :, cs0])
    nc.sync.dma_start(out=wk_v[:, H1], in_=wk8_v[:, H1])
    nc.scalar.dma_start(out=xk_c[0][:, H1], in_=xk_v[:, H1, :, cs0])
    nc.sync.dma_start(out=wk_v[:, H2], in_=wk8_v[:, H2])
    nc.scalar.dma_start(out=xk_c[0][:, H2], in_=xk_v[:, H2, :, cs0])
    nc.sync.dma_start(out=wq_v[:, H1], in_=wq8_v[:, H1])
    nc.scalar.dma_start(out=xq_c[0][:, H1], in_=xq_v[:, H1, :, cs0])
    nc.sync.dma_start(out=wq_v[:, H2], in_=wq8_v[:, H2])
    nc.scalar.dma_start(out=xq_c[0][:, H2], in_=xq_v[:, H2, :, cs0])
    nc.scalar.dma_start(out=ident_t, in_=ident)
    if nvb:
        nc.scalar.dma_start(
            out=valid_sb.rearrange("p (n f) -> p n f", f=BLK),
            in_=validT.rearrange("n p f -> p n f"),
        )
    for ib in range(1, NIB):
        load_x(ib, ib)
    nc.scalar.dma_start(
        out=wo_t.rearrange("p (q e) -> p q e", e=D),
        in_=wo.rearrange("(q p) e -> p q e", p=BLK),
    )

    # ---- compute helpers ---------------------------------------------
    # hi/lo fp8 DoubleRow product: acc += (Ah+Al)' Bh  +  Ah' Bl
    #   A = stationary [p, d, 2, m-cols], B = moving [p, d, 2, n-cols]
    def dr_matmuls(ps, a4, b4, mslice, nslice):
        for d in range(NDT):
            nc.tensor.matmul(
                ps,
                a4[:, d, :, mslice],
                b4[:, d, 0:1, nslice].broadcast_to(
                    (BLK, 2, b4[:, d, 0, nslice].shape[-1])
                ),
                start=(d == 0),
                stop=False,
                perf_mode=DR,
            )
        for d0 in range(0, NDT, 2):
            nc.tensor.matmul(
                ps,
                a4[:, d0:d0 + 2, 0, mslice],
                b4[:, d0:d0 + 2, 1, nslice],
                start=False,
                stop=(d0 == NDT - 2),
                perf_mode=DR,
            )

    def vproj_unit(j):
        sb = j // 4
        jc = slice((j % 4) * BLK, (j % 4 + 1) * BLK)
        ps = projp.tile([BLK, IBW], F32, tag="pp", name=f"vps{j}_{r}")
        wv4 = wv_t.rearrange(w_shape, two=2, f=FB)
        dr_matmuls(ps, xv_c[sb], wv4, jc, slice(0, FB))
        dst = v_sb[j].rearrange("p (h w) -> p h w", w=VW)
        nc.vector.tensor_scalar_mul(
            dst[:, :, 0:HD], ps.rearrange("p (h w) -> p h w", w=HD), INV_SCALE
        )
        nc.vector.memset(dst[:, :, HD:VW], 1.0)

    def project(p, ib, w_t, x_c, dst_sb, nm):
        ps = projp.tile([BLK, IBW], F32, tag="pp", name=nm)
        w4 = w_t.rearrange(w_shape, two=2, f=FB)
        dr_matmuls(ps, w4, x_c[ib], slice(p * BLK, (p + 1) * BLK), slice(0, IBW))
        nc.vector.tensor_scalar_mul(
            dst_sb[p][:, ib * IBW:(ib + 1) * IBW], ps, INV_SCALE
        )

    filler = []
    UNIT_COST = {"ctxT": 1}

    def drain(budget):
        while filler and budget > 0:
            k, fn = filler.pop(0)
            fn()
            budget -= UNIT_COST.get(k, 4)

    def drain_kind(kind):
        keep = []
        for k, fn in filler:
            if k == kind:
                fn()
            else:
                keep.append((k, fn))
        filler[:] = keep

    def attention(p, ib):
        jts = plan[ib]
        nj = len(jts)
        # coverage bookkeeping: which entries touch each 128-i-chunk
        cov = [
            [e for e, (jt, c0, c1, mx) in enumerate(jts) if c0 <= ch * BLK < c1]
            for ch in range(NCH)
        ]
        last_cov = [c[-1] for c in cov]
        # ctx psum: one 1-bank tile per head, [p, chunk(4), 128] with data
        # in [0:65] of each region; bufs=2 double-buffers across head pairs.
        ctx_ps = [
            ctxp.tile([BLK, NCH * CRW], F32, tag="ctx", name=f"c_{p}_{ib}_{h}_{r}")
            for h in range(2)
        ]
        ctx4 = [t.rearrange("p (c v) -> p c v", c=NCH, v=CRW) for t in ctx_ps]
        bank_started = [False, False]
        sc_t = {}

        def emit_scores(e):
            jt, c0, c1, mixed = jts[e]
            w = c1 - c0
            sc = scp.tile([BLK, 2 * IBW], F32, tag="sc", name=f"s{p}_{ib}_{jt}_{r}")
            nc.tensor.matmul(
                sc[:, c0:c1],
                kT_sb[p][0:HD, jt * BLK:(jt + 1) * BLK],
                qT_sb[p][0:HD, ib * IBW + c0:ib * IBW + c1],
                start=True,
                stop=True,
            )
            nc.tensor.matmul(
                sc[:, IBW:IBW + w],
                kT_sb[p][HD:BLK, jt * BLK:(jt + 1) * BLK],
                qT_sb[p][HD:BLK, ib * IBW + c0:ib * IBW + c1],
                start=True,
                stop=True,
                tile_position=(HD, 0),
            )
            for k in mixed:
                slot = mixed_slots[(jt, 4 * ib + k)]
                mv = valid_sb[:, slot * BLK:(slot + 1) * BLK]
                nc.tensor.matmul(
                    sc[:, k * BLK:(k + 1) * BLK],
                    ident_t, mv,
                    start=False, stop=True, skip_group_check=True,
                )
                h1c = IBW + k * BLK - c0
                nc.tensor.matmul(
                    sc[:, h1c:h1c + BLK],
                    ident_t, mv,
                    start=False, stop=True, skip_group_check=True,
                )
            sc_t[e] = sc

        at_t = {}

        def emit_exp(e):
            jt, c0, c1, mixed = jts[e]
            w = c1 - c0
            sc = sc_t.pop(e)
            at = atp.tile([BLK, 2 * IBW], BF, tag="at", name=f"a{p}_{ib}_{jt}_{r}")
            nc.scalar.activation(
                out=at[:, c0:IBW + w],
                in_=sc[:, c0:IBW + w],
                func=mybir.ActivationFunctionType.Exp,
                scale=float(EXP_SCALE),
            )
            at_t[e] = at

        def emit_ctx(e):
            jt, c0, c1, mixed = jts[e]
            at = at_t.pop(e)
            vv = v_sb[jt].rearrange("p (h w) -> p h w", w=VW)
            for h in range(2):
                for ch in range(c0 // BLK, NCH):
                    if h == 0:
                        lhsT = at[:, ch * BLK:(ch + 1) * BLK]
                    else:
                        o = IBW + ch * BLK - c0
                        lhsT = at[:, o:o + BLK]
                    st = not bank_started[h]
                    bank_started[h] = True
                    nc.tensor.matmul(
                        ctx4[h][:, ch, 0:VW],
                        lhsT,
                        vv[:, 2 * p + h, :],
                        start=st,
                        stop=(e == last_cov[ch]),
                    )

        # Per 2-entry group: exps first (registers the sc-slot readers),
        # one filler unit (~independent matmuls) to absorb the exp wait,
        # then the next score pair (WAR on the sc slot), then both ctx
        # batches.
        emit_scores(0)
        if nj > 1:
            emit_scores(1)
        for e0 in range(0, nj, 2):
            es = [e for e in (e0, e0 + 1) if e < nj]
            for e in es:
                emit_exp(e)
            drain(7)
            for e in es:
                if e + 2 < nj:
                    emit_scores(e + 2)
            for e in es:
                emit_ctx(e)

        # evacuation: one strided reciprocal of the 8 denominator columns,
        # then per (head, chunk): per-partition normalize -> bf16 stg ->
        # PE transpose (bitcast view of a projp slot) -> Pool copy into
        # the f-major ctxT tile the out-projection consumes.
        rec = small.tile([BLK, 2 * NCH], F32, tag="rec", name=f"rc{p}_{ib}_{r}")
        rec3 = rec.rearrange("p (h c v) -> p h c v", h=2, c=NCH, v=1)
        for h in range(2):
            nc.vector.reciprocal(out=rec3[:, h], in_=ctx4[h][:, :, HD:VW])
        for h in range(2):
            # one normalize-mul per head: recip column broadcast (stride 0)
            # along hd, all 4 chunks in one op
            stg = stgp.tile([BLK, NCH * HD], BF, tag="stg", name=f"st{p}_{ib}_{h}_{r}")
            stg3 = stg.rearrange("p (c w) -> p c w", w=HD)
            nc.vector.tensor_mul(
                stg3,
                ctx4[h][:, :, 0:HD],
                rec3[:, h].broadcast_to((BLK, NCH, HD)),
            )
            for ch in range(NCH):
                # transpose + copy only read stg, not the ctx psum: defer
                # them into the filler stream so the PE transpose never
                # stalls inline on the DVE normalize.
                def tposeu(stg_=stg3, h_=h, ch_=ch, p_=p, ib_=ib):
                    tp = projp.tile([BLK, IBW], F32, tag="pp",
                                    name=f"tp{p_}_{ib_}_{h_}_{ch_}_{r}")
                    tpv = tp[:, :].bitcast(BF)
                    nc.tensor.transpose(tpv[0:HD, 0:BLK], stg_[:, ch_], ident_t)
                    nc.vector.tensor_copy(
                        ctxT_sb[p_][h_ * HD:(h_ + 1) * HD,
                                    ib_ * IBW + ch_ * BLK:ib_ * IBW + (ch_ + 1) * BLK],
                        tpv[0:HD, 0:BLK],
                    )

                filler.append(("ctxT", tposeu))

    def outproj_mms(po, st, nb, ps):
        for p in ps:
            nc.tensor.matmul(
                po,
                ctxT_sb[p][:, st * BLK:(st + 1) * BLK],
                wo_t[:, p * D + nb * IBW:p * D + (nb + 1) * IBW],
                start=(p == 0),
                stop=(p == NPAIR - 1),
            )

    def outproj_evac(po, st, nb):
        ot = small.tile([BLK, IBW], F32, tag="ot", name=f"ot{st}_{nb}_{r}")
        nc.vector.tensor_copy(ot, po)
        nc.scalar.dma_start(
            out=out[st * BLK:(st + 1) * BLK, nb * IBW:(nb + 1) * IBW],
            in_=ot,
        )

    def outproj_unit(st, nb):
        po = projp.tile([BLK, IBW], F32, tag="pp", name=f"po{st}_{nb}_{r}")
        outproj_mms(po, st, nb, range(NPAIR))
        outproj_evac(po, st, nb)

    def outproj_q(st, nq):
        # quarter-width unit (256 out cols) for the kernel tail
        pof = projp.tile([BLK, IBW], F32, tag="pp", name=f"pq{st}_{nq}_{r}")
        po = pof[:, 0:IBW // 2]
        qc = slice(nq * (IBW // 2), (nq + 1) * (IBW // 2))
        for p in range(NPAIR):
            nc.tensor.matmul(
                po,
                ctxT_sb[p][:, st * BLK:(st + 1) * BLK],
                wo_t[:, p * D:(p + 1) * D][:, qc],
                start=(p == 0),
                stop=(p == NPAIR - 1),
            )
        otf = small.tile([BLK, IBW], F32, tag="ot", name=f"otq{st}_{nq}_{r}")
        ot = otf[:, 0:IBW // 2]
        nc.vector.tensor_copy(ot, po)
        nc.scalar.dma_start(
            out=out[st * BLK:(st + 1) * BLK, qc],
            in_=ot,
        )

    def outproj_halves(st, nb):
        # two 2-matmul filler units sharing one accumulation group; A and
        # B are pushed adjacently so the 2-slot pp rotation never has to
        # wait on a B that sits later in the PE FIFO.
        holder = []

        def ua():
            po = projp.tile([BLK, IBW], F32, tag="pp", name=f"po{st}_{nb}_{r}")
            holder.append(po)
            outproj_mms(po, st, nb, (0, 1))

        def ub():
            po = holder[0]
            outproj_mms(po, st, nb, (2, 3))
            outproj_evac(po, st, nb)

        return ua, ub

    # ---- main schedule ------------------------------------------------
    # vproj(ib0) runs inline before the first attention; later vprojs and
    # all out-projections flow through the filler queue, drained one unit
    # per 2-entry attention group (PE bubble absorption). vproj units are
    # force-drained at each ib boundary (needed by the next ib's ctx).
    for j in range(4):
        vproj_unit(j)
    for ib in range(NIB):
        for p in range(NPAIR):
            if p == 2 and ib + 1 < NIB:
                for j in range(4 * (ib + 1), 4 * (ib + 1) + 4):
                    filler.append(("vproj", (lambda jj: lambda: vproj_unit(jj))(j)))
            if p == 1 and ib > 0:
                for st in range(4 * (ib - 1), 4 * (ib - 1) + 4):
                    for nb in range(2):
                        if ib == NIB - 1:
                            # ib3 has 32 drain slots but little supply:
                            # split its outproj fillers into halves
                            ua, ub = outproj_halves(st, nb)
                            filler.append(("outproj", ua))
                            filler.append(("outproj", ub))
                        else:
                            filler.append(
                                ("outproj", (lambda s_, n_: lambda: outproj_unit(s_, n_))(st, nb))
                            )
            if p == 0 and ib == 0:
                project(0, 0, wk_t, xk_c, kT_sb, f"kps0_0_{r}")
                project(0, 0, wq_t, xq_c, qT_sb, f"qps0_0_{r}")
            # queue next pair's k/q projections as filler (emitted inside
            # this attention; force-drained at its end)
            np_, nib = (p + 1, ib) if p + 1 < NPAIR else (0, ib + 1)
            if nib < NIB:
                for w_t, x_c, dst, nm in (
                    (wk_t, xk_c, kT_sb, f"kps{np_}_{nib}_{r}"),
                    (wq_t, xq_c, qT_sb, f"qps{np_}_{nib}_{r}"),
                ):
                    # deadline-critical: next pair's scores stall on these,
                    # so they jump the filler queue
                    filler.insert(
                        len([u for u in filler if u[0] == "kqps"]),
                        ("kqps", (lambda a, b, c, d, e2, f2: lambda: project(a, b, c, d, e2, f2))(
                            np_, nib, w_t, x_c, dst, nm))
                    )
            attention(p, ib)
            drain_kind("kqps")
        drain_kind("vproj")
    drain_kind("fin")
    drain_kind("ctxT")
    drain_kind("outproj")
    for st in range(4 * (NIB - 1), 4 * (NIB - 1) + 4):
        for nb in range(2):
            outproj_unit(st, nb)



_BUILD_CACHE: dict = {}


def _get_nc(mask: np.ndarray, reps=1):
    states = classify_mask(mask)
    plan, mixed_slots = build_plan(states)
    key = (plan_key(plan, mixed_slots), reps)
    if key not in _BUILD_CACHE:
        _BUILD_CACHE[key] = (build_nc(plan, mixed_slots, reps), plan, mixed_slots)
    return _BUILD_CACHE[key]


def _hilo_dr(a, scale):
    """[R, C] f32 -> fp8 hi/lo interleaved [128, (R//128)*2*C]."""
    a = np.asarray(a, np.float32) * scale
    a = a.reshape(-1, BLK, a.shape[-1])            # [d, p, C]
    hi = np.clip(a, -240, 240).astype(F8)
    lo = np.clip(a - hi.astype(np.float32), -240, 240).astype(F8)
    arr = np.stack([hi, lo], axis=2)               # [d, p, 2, C]
    return np.ascontiguousarray(arr.transpose(1, 0, 2, 3)).reshape(BLK, -1)


def _make_in_maps(xq, xk, xv, mask, W_q, W_k, W_v, W_o, mixed_slots):
    nvb = max(1, len(mixed_slots))
    vt = np.zeros((nvb, BLK, BLK), BF16)
    m = np.asarray(mask)
    for (jt, it), slot in mixed_slots.items():
        blk = m[it * BLK:(it + 1) * BLK, jt * BLK:(jt + 1) * BLK]
        vt[slot] = (-30000.0 * blk.T.astype(np.float32)).astype(BF16)
    ident = np.eye(BLK, dtype=BF16)
    xT = {}
    for b in range(B):
        xT[b] = tuple(
            _hilo_dr(np.asarray(x[b]).T, SX) for x in (xq, xk, xv)
        )
    wmaps = {}
    for g in range(G):
        cols = slice(g * FB, (g + 1) * FB)
        wmaps[g] = (
            _hilo_dr(np.asarray(W_q)[:, cols], SW),
            _hilo_dr(np.asarray(W_k)[:, cols], SW),
            _hilo_dr(np.asarray(W_v)[:, cols], SW),
            np.asarray(W_o)[cols, :].astype(BF16),
        )
    in_maps = []
    for c in range(8):
        b, g = c // G, c % G
        in_maps.append(
            {
                "xq8": xT[b][0],
                "xk8": xT[b][1],
                "xv8": xT[b][2],
                "wq8": wmaps[g][0],
                "wk8": wmaps[g][1],
                "wv8": wmaps[g][2],
                "wo": wmaps[g][3],
                "ident": ident,
                "validT": vt,
            }
        )
    return in_maps


PROFILE = False
last_hw_exec_ns = None


def kernel(xq, xk, xv, mask, W_q, W_k, W_v, W_o):
    global last_hw_exec_ns
    from concourse import bass_utils

    nc, plan, mixed_slots = _get_nc(mask)
    in_maps = _make_in_maps(xq, xk, xv, mask, W_q, W_k, W_v, W_o, mixed_slots)
    res = bass_utils.run_bass_kernel_spmd(
        nc, in_maps, core_ids=list(range(8))
    )
    if res.exec_time_ns:
        last_hw_exec_ns = res.exec_time_ns
    out = np.empty((B, S, D), np.float32)
    for b in range(B):
        out[b] = res.results[2 * b]["out"] + res.results[2 * b + 1]["out"]
    return out
